# revision 42
# baseline (speedup 1.0000x reference)
"""CRF token-classifier loss (nn_CRFTokenClassifier) on 8 Trainium2 NeuronCores.

v5 strategy (data-parallel over batch, 8 sequences per core):
  - hidden staged fp8 (e4m3) in DoubleRow layout on the sync ring;
    emissions^T = (W*64)^T @ hidden^T as 24 dual-fp8 matmuls (K_eff=256,
    216ns/matmul at full clock). A ~6us dummy-weight PE warmup starting at
    preamble end lifts the HAM clock gate before the first pair lands
    (measured: first 14 matmuls run 630ns at half clock otherwise).
  - PSUM descaled (1/64) to SBUF bf16 by scalar+vector alternating;
    DRAM bounce (write then read, both on the scalar ring FIFO) into tree
    layout emt[p=(seq,chunk), l, 32] bf16.
  - log-partition on the vector engine in bf16 (2x DVE throughput): L0
    makes 16 pair records per partition in 3 instructions from a
    host-baked per-pair bf16 table exp(T_ij+b_j+T_jk+b_k);
    start_transitions folded into pair 0 of seq-start partitions,
    end_transitions into pair 15 of seq-end partitions; binary levels as
    3 independent strided mults + one X-reduce (4 instrs, one dependence
    hop); one max-normalize whose ln(max) rides the gold PE-gather
    (2-col rhs); single pack DMA to partitions 0-7; tail levels the same
    4-instr shape with the last level XY-reducing row 0 straight to Z.
  - gold score: host-baked bf16 one-hot labels, Pool mult + scalar
    accum-copy (f32 accum), summed per-seq by the PE gather; transition/
    start/end/bias terms are one host f32 scalar per sequence.
  - attention_mask is all ones by construction (fill: ones); masked-step
    handling is omitted like the baseline.
  - per-core output: per-sequence (logZ - score); host sums / B.
"""

import sys

if "/opt/trn_rl_repo" not in sys.path:
    sys.path.insert(0, "/opt/trn_rl_repo")

import numpy as np
import ml_dtypes

B, S, H, L = 64, 512, 768, 3
NCORES = 8
BC = B // NCORES            # 8 sequences (blocks) per core
NCH = 16                    # 32-step chunks per sequence
TS = 32
NPAIR = 4                   # block pairs
WSCALE = 64.0               # fp8 weight scale
NSC = 3                     # DoubleRow super-chunks (256 h each)

# bf16 const tensor column layout ([128, NCB])
CPAIR = 0                   # pair tables [u][j][(i,k)] : 16*27
COH = 432                   # one-hot labels [l,t] layout, 96 cols
NCB = 528
# f32 const tensor column layout ([128, NCF])
CSEL = 0                    # 8 cols: per-seq gather indicator
CGC = 8                     # 1 col: per-seq gold const at partitions 0-7
NCF = 9


def _patch_act_tables(arch):
    """Steer the act-table chooser so every activation we use resolves to the
    combined exp+ln set: one ACT_TABLE_LOAD for Copy / Exp / Ln."""
    from concourse.hw_specs import get_activation_tables
    from concourse import mybir

    A = mybir.ActivationFunctionType
    tabs = get_activation_tables(arch)
    combined = None
    for name, fns in tabs.items():
        if A.Exp in fns and A.Ln in fns:
            combined = name
            break
    if combined is None:
        return
    for f in (A.Exp, A.Ln, A.Copy, A.Identity):
        if f not in tabs[combined]:
            continue
        for name, fns in tabs.items():
            if name != combined:
                fns.discard(f)


def _build_nc(debug=False):
    import concourse.bass as bass
    import concourse.bacc as bacc
    import concourse.tile as tile
    from concourse import mybir

    f32 = mybir.dt.float32
    bf16 = mybir.dt.bfloat16
    fp8 = mybir.dt.float8e4
    Alu = mybir.AluOpType
    Act = mybir.ActivationFunctionType
    AX = mybir.AxisListType
    DR = mybir.MatmulPerfMode.DoubleRow

    nc = bacc.Bacc(None, target_bir_lowering=False, debug=debug)
    _patch_act_tables(nc.m.arch)

    hs_d = nc.dram_tensor("hseq", [BC, 128, NSC * 1024], fp8,
                          kind="ExternalInput")
    # stationary padded to 16 cols: dual-fp8 Ldweights ISA check rejects
    # narrow weight tiles (M=3/4 fail, M=16 passes)
    w_d = nc.dram_tensor("w8", [128, NSC * 32], fp8, kind="ExternalInput")
    cb_d = nc.dram_tensor("constb", [128, NCB], bf16, kind="ExternalInput")
    cf_d = nc.dram_tensor("constf", [128, NCF], f32, kind="ExternalInput")
    out = nc.dram_tensor("diff", [BC, 1], f32, kind="ExternalOutput")

    em_ds = [nc.dram_tensor(f"em_scratch{q}", [L, 1024], bf16)
             for q in range(NPAIR)]

    def sl(tile_h, pb, nparts, extra, dims):
        """AP over a tile's partitions [pb, pb+nparts), free-dim pattern
        `dims`, extra element offset `extra`."""
        ap = tile_h[:]
        return bass.AP(tile_h.tensor, ap.offset + pb * ap.ap[0][0] + extra,
                       [[ap.ap[0][0], nparts]] + dims)

    with tile.TileContext(nc) as tc:
        with (
            tc.tile_pool(name="consts", bufs=1) as cp,
            tc.tile_pool(name="hload", bufs=1) as hp,
            tc.tile_pool(name="emx", bufs=2) as ep,
            tc.tile_pool(name="tree", bufs=1) as rp,
            tc.tile_pool(name="gold", bufs=1) as gp,
            tc.tile_pool(name="pe", bufs=4, space="PSUM") as pep,
            tc.tile_pool(name="ps", bufs=1, space="PSUM") as psp,
        ):
            V = nc.vector
            G = nc.gpsimd

            # ---- PE warmup on memset dummy data (no DMA gate): the HAM
            # clock gate tracks ~6us of SUBSTANTIAL matmul work (tiny
            # matmuls don't move it), so run 9 full-width dummy matmuls
            # before pair 0 lands ----
            wdum = cp.tile([128, 16], fp8)
            wdumB = cp.tile([128, 512], fp8)
            G.memset(sl(wdum, 0, 128, 0, [[1, 16]]), 0.25)
            V.memset(sl(wdumB, 0, 128, 0, [[1, 512]]), 0.25)
            pwarm = psp.tile([16, 512], f32, name="pwarm")
            for _ in range(15):
                nc.tensor.matmul(pwarm[:], wdum[:], wdumB[:],
                                 start=True, stop=True)

            # ---- preloads: w8 first on the sync ring, then the hidden
            # pairs; const tensors on the scalar ring ----
            wsb = cp.tile([128, NSC, 2, 16], fp8)
            nc.sync.dma_start(
                wsb[:], w_d[:].rearrange("p (s i l) -> p s i l", i=2, l=16))
            cb = cp.tile([128, NCB], bf16)
            nc.scalar.dma_start(cb[:], cb_d[:])
            cf = cp.tile([128, NCF], f32)
            nc.scalar.dma_start(cf[:], cf_d[:])

            def cbsl(pb, nparts, col, dims):
                return sl(cb, pb, nparts, col, dims)

            def cfsl(pb, nparts, col, dims):
                return sl(cf, pb, nparts, col, dims)

            # hidden loads split 1+3 across two queues: pair 0 on the sync
            # ring, pairs 1-3 on the Pool ring — both pull HBM concurrently
            # so the last pair lands ~2us earlier and pair 0 arrives while
            # the warmup still runs
            hs = hp.tile([128, BC, NSC * 1024], fp8)
            nc.sync.dma_start(
                hs[:, 0:2, :], hs_d[0:2].rearrange("b p x -> p b x"))
            for q in range(1, NPAIR):
                G.dma_start(
                    hs[:, 2 * q:2 * q + 2, :],
                    hs_d[2 * q:2 * q + 2].rearrange("b p x -> p b x"))

            packT = gp.tile([128, 16, 9], bf16, name="packT")

            # ---- emissions: per pair, 6 DoubleRow matmuls (+12 filler
            # warmups to hold the clock through DMA gaps); the two descale
            # copies run on scalar+vector in parallel; bounce write on the
            # scalar ring (program order after its copy), read on the Pool
            # ring (pre-issued descriptor, waits the write semaphore) ----
            emt = rp.tile([128, L, TS], bf16)   # [p=(seq,chunk), l, t]
            for q in range(NPAIR):
                emb = ep.tile([L, 2, 512], bf16, tag="emb")
                for i in range(2):
                    b = 2 * q + i
                    pe = pep.tile([16, 512], f32, tag="pe")
                    for sc in range(NSC):
                        nc.tensor.matmul(
                            pe[:],
                            wsb[:, sc, :, :],
                            sl(hs, 0, 128, b * (NSC * 1024) + sc * 1024,
                               [[512, 2], [1, 512]]),
                            start=(sc == 0), stop=(sc == NSC - 1),
                            perf_mode=DR)
                    if i == 0:
                        nc.scalar.mul(emb[:, i, :], pe[0:L, :], 1.0 / WSCALE)
                    else:
                        V.tensor_scalar_mul(emb[:, i, :], pe[0:L, :],
                                            1.0 / WSCALE)
                nc.scalar.dma_start(
                    bass.AP(em_ds[q], 0, [[1024, L], [1, 1024]]), emb[:])
                nc.scalar.dma_start(
                    sl(emt, 32 * q, 32, 0, [[TS, L], [1, TS]]),
                    bass.AP(em_ds[q], 0, [[TS, 32], [1024, L], [1, TS]]))

            ga2 = gp.tile([128, 2], f32, name="ga2")  # [gold, ln-offset]

            # ---- exp of emissions (biases folded into the pair tables) ----
            em_e = rp.tile([128, L, TS], bf16)
            nc.scalar.activation(
                sl(em_e, 0, 128, 0, [[1, L * TS]]),
                sl(emt, 0, 128, 0, [[1, L * TS]]),
                Act.Exp)

            # ---- L0: j-mults on V (j=0,1) and Pool (j=2), adds + eb on V ----
            ta0 = rp.tile([128, 16, 9], bf16, name="ta0")
            tb0 = rp.tile([128, 16, 9], bf16, name="tb0")
            tg0 = rp.tile([128, 16, 9], bf16, name="tg0")
            r0 = rp.tile([128, 16, 9], bf16, name="r0")
            l0sl = lambda t: sl(t, 0, 128, 0, [[9, 16], [3, 3], [1, 3]])
            tab = lambda j: cbsl(0, 128, CPAIR + 9 * j,
                                 [[27, 16], [3, 3], [1, 3]])
            ea = lambda j: sl(em_e, 0, 128, 2 * 0 + TS * j,
                              [[2, 16], [0, 3], [0, 3]])
            G.tensor_mul(l0sl(tg0), tab(2), ea(2))
            V.tensor_mul(l0sl(ta0), tab(0), ea(0))
            V.tensor_mul(l0sl(tb0), tab(1), ea(1))
            V.tensor_add(l0sl(ta0), l0sl(ta0), l0sl(tb0))
            V.tensor_add(l0sl(ta0), l0sl(ta0), l0sl(tg0))
            V.tensor_mul(
                sl(r0, 0, 128, 0, [[9, 16], [3, 3], [1, 3]]),
                l0sl(ta0),
                sl(em_e, 0, 128, 1, [[2, 16], [0, 3], [TS, 3]]))

            # ---- binary fold: j-mults on V (j=0,1) and Pool (j=2), then one
            # V X-reduce over j (one dependence hop per level) ----
            def fold(cur, stride, pb, np_, n, tmp, nxt):
                half = n // 2
                A = lambda j: sl(cur, pb, np_, j,
                                 [[2 * stride, half], [3, 3], [0, 3]])
                Bp = lambda j: sl(cur, pb, np_, stride + 3 * j,
                                  [[2 * stride, half], [0, 3], [1, 3]])
                G.tensor_mul(
                    sl(tmp, pb, np_, 2, [[27, half], [9, 3], [3, 3]]),
                    A(2), Bp(2))
                for j in range(2):
                    V.tensor_mul(
                        sl(tmp, pb, np_, j, [[27, half], [9, 3], [3, 3]]),
                        A(j), Bp(j))
                V.tensor_reduce(
                    sl(nxt, pb, np_, 0, [[9, half], [1, 9]]),
                    sl(tmp, pb, np_, 0, [[27, half], [3, 9], [1, 3]]),
                    axis=AX.X, op=Alu.add)

            lv8 = rp.tile([128, 8, 9], bf16, name="lv8")
            lv4 = rp.tile([128, 4, 9], bf16, name="lv4")
            lv2 = rp.tile([128, 2, 9], bf16, name="lv2")
            tmp8 = rp.tile([128, 8, 27], bf16, name="tmp8")
            r1 = rp.tile([128, 9], f32, name="r1")
            Sm = rp.tile([128, 27], bf16, name="Sm")

            with nc.allow_low_precision("bf16 tree folds (3-term adds)"):
                fold(r0, 9, 0, 128, 16, tmp8, lv8)
                fold(lv8, 9, 0, 128, 8, tmp8, lv4)
                fold(lv4, 9, 0, 128, 4, tmp8, lv2)
            # last in-partition fold via mult + X-reduce (f32 out)
            V.tensor_mul(
                sl(Sm, 0, 128, 0, [[9, 3], [3, 3], [1, 3]]),
                sl(lv2, 0, 128, 0, [[3, 3], [0, 3], [1, 3]]),
                sl(lv2, 0, 128, 9, [[0, 3], [1, 3], [3, 3]]))
            V.tensor_reduce(
                sl(r1, 0, 128, 0, [[3, 3], [1, 3]]),
                sl(Sm, 0, 128, 0, [[9, 3], [3, 3], [1, 3]]),
                axis=AX.X, op=Alu.add)

            # ---- normalize; ln(max) goes to the gather tile col 1 ----
            mx = rp.tile([128, 1], f32, name="mx")
            rinv = rp.tile([128, 1], f32, name="rinv")
            rec9 = rp.tile([128, 9], bf16, name="rec9")
            V.tensor_reduce(sl(mx, 0, 128, 0, [[1, 1]]),
                            sl(r1, 0, 128, 0, [[1, 9]]),
                            axis=AX.X, op=Alu.max)
            V.reciprocal(sl(rinv, 0, 128, 0, [[1, 1]]),
                         sl(mx, 0, 128, 0, [[1, 1]]))
            V.tensor_mul(
                sl(rec9, 0, 128, 0, [[1, 9]]),
                sl(r1, 0, 128, 0, [[1, 9]]),
                sl(rinv, 0, 128, 0, [[0, 9]]))
            nc.scalar.activation(sl(ga2, 0, 128, 1, [[1, 1]]),
                                 sl(mx, 0, 128, 0, [[1, 1]]), Act.Ln)

            # ---- gold emission-sum: Pool mult, scalar accum-copy ----
            dsc = gp.tile([128, L * TS], bf16, name="dsc")
            G.tensor_mul(
                sl(dsc, 0, 128, 0, [[1, L * TS]]),
                cbsl(0, 128, COH, [[1, L * TS]]),
                sl(emt, 0, 128, 0, [[1, L * TS]]))
            nc.scalar.activation(
                sl(dsc, 0, 128, 0, [[1, L * TS]]),
                sl(dsc, 0, 128, 0, [[1, L * TS]]),
                Act.Copy, accum_out=sl(ga2, 0, 128, 0, [[1, 1]]))

            # ---- per-seq gather: gold sum + ln-offset sum in one matmul ----
            sc8 = psp.tile([BC, 2], f32, name="sc8")
            nc.tensor.matmul(sc8[:], cfsl(0, 128, CSEL, [[1, BC]]),
                             sl(ga2, 0, 128, 0, [[1, 2]]),
                             start=True, stop=True)

            # ---- pack 16 chunk records per seq to partitions 0-7 (Pool
            # ring: the issue happens early, the descriptor waits) ----
            G.dma_start(
                sl(packT, 0, BC, 0, [[1, 144]]),
                sl(rec9, 0, 128, 0, [[1, 9]]))

            # ---- tail tree 16 -> 1 per seq; last level XY-reduces row 0
            # straight to Z ----
            t8 = gp.tile([128, 8, 9], bf16, name="t8")
            t4 = gp.tile([128, 4, 9], bf16, name="t4")
            t2 = gp.tile([128, 2, 9], bf16, name="t2")
            SmF = gp.tile([128, 9], bf16, name="SmF")
            zs = gp.tile([128, 1], f32, name="zs")
            with nc.allow_low_precision("bf16 tail folds (3-term adds)"):
                fold(packT, 9, 0, BC, 16, tmp8, t8)
                fold(t8, 9, 0, BC, 8, tmp8, t4)
                fold(t4, 9, 0, BC, 4, tmp8, t2)
            # zs = sum_{j,k} t2[rec0][0,j] * t2[rec1][j,k]  (end_transitions
            # folded into the last pair table on seq-end partitions)
            V.tensor_mul(
                sl(SmF, 0, BC, 0, [[3, 3], [1, 3]]),
                sl(t2, 0, BC, 0, [[1, 3], [0, 3]]),
                sl(t2, 0, BC, 9, [[3, 3], [1, 3]]))
            V.tensor_reduce(
                sl(zs, 0, BC, 0, [[1, 1]]),
                sl(SmF, 0, BC, 0, [[3, 3], [1, 3]]),
                axis=AX.XY, op=Alu.add)

            # ---- finals ----
            lz = gp.tile([128, 1], f32, name="lz")
            logz = gp.tile([128, 1], f32, name="logz")
            diffT = gp.tile([128, 1], f32, name="diffT")
            nc.scalar.activation(sl(lz, 0, BC, 0, [[1, 1]]),
                                 sl(zs, 0, BC, 0, [[1, 1]]), Act.Ln)
            V.tensor_add(sl(logz, 0, BC, 0, [[1, 1]]),
                         sl(lz, 0, BC, 0, [[1, 1]]),
                         sl(sc8, 0, BC, 1, [[1, 1]]))
            V.scalar_tensor_tensor(
                sl(diffT, 0, BC, 0, [[1, 1]]),
                sl(logz, 0, BC, 0, [[1, 1]]),
                cfsl(0, BC, CGC, [[1, 1]]),
                sl(sc8, 0, BC, 0, [[1, 1]]),
                Alu.subtract, Alu.subtract)
            G.dma_start(out[:], sl(diffT, 0, BC, 0, [[1, 1]]))

    nc.compile()
    return nc


_NC_CACHE = {}


def get_nc(debug=False):
    if "nc" not in _NC_CACHE:
        _NC_CACHE["nc"] = _build_nc(debug)
    return _NC_CACHE["nc"]


def make_in_maps(hidden, W, b, start_transitions, end_transitions, transitions,
                 attention_mask, labels):
    hidden = np.asarray(hidden, dtype=np.float32)
    W = np.asarray(W, dtype=np.float32)
    bb = np.asarray(b, dtype=np.float32)
    st = np.asarray(start_transitions, dtype=np.float32)
    en = np.asarray(end_transitions, dtype=np.float32)
    tr = np.asarray(transitions, dtype=np.float32)
    lab = np.asarray(labels)
    lab = np.where(lab < 0, 0, lab).astype(np.int64)

    e4 = ml_dtypes.float8_e4m3
    b16 = ml_dtypes.bfloat16
    # w8[p, (sc, i, l)] = W[sc*256 + i*128 + p, l] * 64, l padded 3 -> 16
    w8f = np.zeros((NSC, 2, 128, 16), dtype=np.float32)
    w8f[:, :, :, :L] = (W * WSCALE).reshape(NSC, 2, 128, L)
    w8 = np.ascontiguousarray(
        w8f.transpose(2, 0, 1, 3).reshape(128, NSC * 32)).astype(e4)

    baseb = np.zeros((128, NCB), dtype=np.float32)
    # pair tables [u][j][(i,k)]: exp(T_ij + b_j + T_jk + b_k); pair 0 on
    # seq-start partitions folds start_transitions (i-replicated); pair 15
    # on seq-end partitions folds end_transitions into the k leg
    u1 = np.exp(tr[:, :, None] + bb[None, :, None]
                + tr.T[None, :, :] + bb[None, None, :])       # [i, j, k]
    u1 = np.ascontiguousarray(u1.transpose(1, 0, 2)).reshape(27)  # [j,(i,k)]
    u0 = np.exp(st[:, None] + bb[:, None] + tr + bb[None, :])     # [j, k]
    u0 = np.broadcast_to(u0[:, None, :], (3, 3, 3)).reshape(27)   # i-bcast
    uE = np.exp(tr[:, :, None] + bb[None, :, None]
                + tr.T[None, :, :] + bb[None, None, :]
                + en[None, None, :])
    uE = np.ascontiguousarray(uE.transpose(1, 0, 2)).reshape(27)
    baseb[:, CPAIR:CPAIR + 432] = np.tile(u1, 16)
    baseb[0::NCH, CPAIR:CPAIR + 27] = u0
    baseb[NCH - 1::NCH, CPAIR + 27 * 15:CPAIR + 432] = uE

    basef = np.zeros((128, NCF), dtype=np.float32)
    p = np.arange(128)
    basef[:, CSEL:CSEL + BC] = (p[:, None] // NCH == np.arange(BC)[None, :])

    in_maps = []
    for c in range(NCORES):
        hc = hidden[c * BC:(c + 1) * BC]                      # [8, 512, 768]
        # hs[b, p, (sc, i, t)] = hidden[b, t, sc*256 + i*128 + p]
        hseq = np.ascontiguousarray(
            hc.reshape(BC, S, NSC, 2, 128).transpose(0, 4, 2, 3, 1)
        ).astype(e4).reshape(BC, 128, NSC * 1024)

        cbc = baseb.copy()
        labc = lab[c * BC:(c + 1) * BC]                       # [8, 512]
        # one-hot in tree layout [p=(s,c), l, t]
        labr = labc.reshape(128, TS)
        cbc[:, COH:COH + L * TS] = (
            labr[:, None, :] == np.arange(L)[None, :, None]
        ).astype(np.float32).reshape(128, L * TS)

        cfc = basef.copy()
        # per-seq gold constant: start + end + transition path + biases
        gc = (st[labc[:, 0]] + en[labc[:, -1]]
              + tr[labc[:, :-1], labc[:, 1:]].sum(axis=1)
              + bb[labc].sum(axis=1))
        cfc[:BC, CGC] = gc

        in_maps.append({
            "hseq": hseq,
            "w8": w8,
            "constb": cbc.astype(b16),
            "constf": cfc,
        })
    return in_maps


def kernel(hidden, W, b, start_transitions, end_transitions, transitions,
           attention_mask, labels):
    from concourse.bass_utils import run_bass_kernel_spmd

    nc = get_nc()
    in_maps = make_in_maps(hidden, W, b, start_transitions, end_transitions,
                           transitions, attention_mask, labels)
    res = run_bass_kernel_spmd(nc, in_maps, core_ids=list(range(NCORES)))
    total = 0.0
    for c in range(NCORES):
        total += float(res.results[c]["diff"].sum())
    return np.float32(total / B)


# revision 46
# speedup vs baseline: 1.0504x; 1.0504x over previous
"""CRF token-classifier loss (nn_CRFTokenClassifier) on 8 Trainium2 NeuronCores.

v5 strategy (data-parallel over batch, 8 sequences per core):
  - hidden staged fp8 (e4m3) in DoubleRow layout on the sync ring;
    emissions^T = (W*64)^T @ hidden^T as 24 dual-fp8 matmuls (K_eff=256,
    216ns/matmul at full clock). A ~6us dummy-weight PE warmup starting at
    preamble end lifts the HAM clock gate before the first pair lands
    (measured: first 14 matmuls run 630ns at half clock otherwise).
  - PSUM descaled (1/64) to SBUF bf16 by scalar+vector alternating;
    DRAM bounce (write then read, both on the scalar ring FIFO) into tree
    layout emt[p=(seq,chunk), l, 32] bf16.
  - log-partition on the vector engine in bf16 (2x DVE throughput): L0
    makes 16 pair records per partition in 3 instructions from a
    host-baked per-pair bf16 table exp(T_ij+b_j+T_jk+b_k);
    start_transitions folded into pair 0 of seq-start partitions,
    end_transitions into pair 15 of seq-end partitions; binary levels as
    3 independent strided mults + one X-reduce (4 instrs, one dependence
    hop); one max-normalize whose ln(max) rides the gold PE-gather
    (2-col rhs); single pack DMA to partitions 0-7; tail levels the same
    4-instr shape with the last level XY-reducing row 0 straight to Z.
  - gold score: host-baked bf16 one-hot labels, Pool mult + scalar
    accum-copy (f32 accum), summed per-seq by the PE gather; transition/
    start/end/bias terms are one host f32 scalar per sequence.
  - attention_mask is all ones by construction (fill: ones); masked-step
    handling is omitted like the baseline.
  - per-core output: per-sequence (logZ - score); host sums / B.
"""

import sys

if "/opt/trn_rl_repo" not in sys.path:
    sys.path.insert(0, "/opt/trn_rl_repo")

import numpy as np
import ml_dtypes

B, S, H, L = 64, 512, 768, 3
NCORES = 8
BC = B // NCORES            # 8 sequences (blocks) per core
NCH = 16                    # 32-step chunks per sequence
TS = 32
NPAIR = 4                   # block pairs
WSCALE = 64.0               # fp8 weight scale
NSC = 3                     # DoubleRow super-chunks (256 h each)

# bf16 const tensor column layout ([128, NCB])
CPAIR = 0                   # pair tables [u][j][(i,k)] : 16*27
COH = 432                   # one-hot labels [l,t] layout, 96 cols
NCB = 528
# f32 const tensor column layout ([128, NCF])
CSEL = 0                    # 8 cols: per-seq gather indicator
CGC = 8                     # 1 col: per-seq gold const at partitions 0-7
NCF = 9


def _patch_act_tables(arch):
    """Steer the act-table chooser so every activation we use resolves to the
    combined exp+ln set: one ACT_TABLE_LOAD for Copy / Exp / Ln."""
    from concourse.hw_specs import get_activation_tables
    from concourse import mybir

    A = mybir.ActivationFunctionType
    tabs = get_activation_tables(arch)
    combined = None
    for name, fns in tabs.items():
        if A.Exp in fns and A.Ln in fns:
            combined = name
            break
    if combined is None:
        return
    for f in (A.Exp, A.Ln, A.Copy, A.Identity):
        if f not in tabs[combined]:
            continue
        for name, fns in tabs.items():
            if name != combined:
                fns.discard(f)


def _build_nc(debug=False):
    import concourse.bass as bass
    import concourse.bacc as bacc
    import concourse.tile as tile
    from concourse import mybir

    f32 = mybir.dt.float32
    bf16 = mybir.dt.bfloat16
    fp8 = mybir.dt.float8e4
    Alu = mybir.AluOpType
    Act = mybir.ActivationFunctionType
    AX = mybir.AxisListType
    DR = mybir.MatmulPerfMode.DoubleRow

    nc = bacc.Bacc(None, target_bir_lowering=False, debug=debug)
    _patch_act_tables(nc.m.arch)

    hs_d = nc.dram_tensor("hseq", [BC, 128, NSC * 1024], fp8,
                          kind="ExternalInput")
    # stationary padded to 16 cols: dual-fp8 Ldweights ISA check rejects
    # narrow weight tiles (M=3/4 fail, M=16 passes)
    w_d = nc.dram_tensor("w8", [128, NSC * 32], fp8, kind="ExternalInput")
    cb_d = nc.dram_tensor("constb", [128, NCB], bf16, kind="ExternalInput")
    cf_d = nc.dram_tensor("constf", [128, NCF], f32, kind="ExternalInput")
    out = nc.dram_tensor("diff", [BC, 1], f32, kind="ExternalOutput")

    em_ds = [nc.dram_tensor(f"em_scratch{q}", [L, 1024], bf16)
             for q in range(NPAIR)]

    def sl(tile_h, pb, nparts, extra, dims):
        """AP over a tile's partitions [pb, pb+nparts), free-dim pattern
        `dims`, extra element offset `extra`."""
        ap = tile_h[:]
        return bass.AP(tile_h.tensor, ap.offset + pb * ap.ap[0][0] + extra,
                       [[ap.ap[0][0], nparts]] + dims)

    with tile.TileContext(nc) as tc:
        with (
            tc.tile_pool(name="consts", bufs=1) as cp,
            tc.tile_pool(name="hload", bufs=1) as hp,
            tc.tile_pool(name="emx", bufs=2) as ep,
            tc.tile_pool(name="tree", bufs=1) as rp,
            tc.tile_pool(name="gold", bufs=1) as gp,
            tc.tile_pool(name="pe", bufs=4, space="PSUM") as pep,
            tc.tile_pool(name="ps", bufs=1, space="PSUM") as psp,
        ):
            V = nc.vector
            G = nc.gpsimd

            # ---- PE warmup on memset dummy data (no DMA gate): the HAM
            # clock gate tracks ~6us of SUBSTANTIAL matmul work (tiny
            # matmuls don't move it), so run 9 full-width dummy matmuls
            # before pair 0 lands ----
            wdum = cp.tile([128, 16], fp8)
            wdumB = cp.tile([128, 512], fp8)
            G.memset(sl(wdum, 0, 128, 0, [[1, 16]]), 0.25)
            V.memset(sl(wdumB, 0, 128, 0, [[1, 512]]), 0.25)
            pwarm = psp.tile([16, 512], f32, name="pwarm")
            for _ in range(10):
                nc.tensor.matmul(pwarm[:], wdum[:], wdumB[:],
                                 start=True, stop=True)

            # ---- preloads: w8 first on the sync ring, then the hidden
            # pairs; const tensors on the scalar ring ----
            wsb = cp.tile([128, NSC, 2, 16], fp8)
            nc.sync.dma_start(
                wsb[:], w_d[:].rearrange("p (s i l) -> p s i l", i=2, l=16))
            cb = cp.tile([128, NCB], bf16)
            nc.scalar.dma_start(cb[:], cb_d[:])
            cf = cp.tile([128, NCF], f32)
            nc.scalar.dma_start(cf[:], cf_d[:])

            def cbsl(pb, nparts, col, dims):
                return sl(cb, pb, nparts, col, dims)

            def cfsl(pb, nparts, col, dims):
                return sl(cf, pb, nparts, col, dims)

            hs = hp.tile([128, BC, NSC * 1024], fp8)
            for q in range(NPAIR):
                nc.sync.dma_start(
                    hs[:, 2 * q:2 * q + 2, :],
                    hs_d[2 * q:2 * q + 2].rearrange("b p x -> p b x"))

            packT = gp.tile([128, 16, 9], bf16, name="packT")

            # ---- emissions: per pair, 6 DoubleRow matmuls (+12 filler
            # warmups to hold the clock through DMA gaps); the two descale
            # copies run on scalar+vector in parallel; bounce write on the
            # scalar ring (program order after its copy), read on the Pool
            # ring (pre-issued descriptor, waits the write semaphore) ----
            emt = rp.tile([128, L, TS], bf16)   # [p=(seq,chunk), l, t]
            for q in range(NPAIR):
                emb = ep.tile([L, 2, 512], bf16, tag="emb")
                for i in range(2):
                    b = 2 * q + i
                    pe = pep.tile([16, 512], f32, tag="pe")
                    for sc in range(NSC):
                        nc.tensor.matmul(
                            pe[:],
                            wsb[:, sc, :, :],
                            sl(hs, 0, 128, b * (NSC * 1024) + sc * 1024,
                               [[512, 2], [1, 512]]),
                            start=(sc == 0), stop=(sc == NSC - 1),
                            perf_mode=DR)
                    if i == 0:
                        nc.scalar.mul(emb[:, i, :], pe[0:L, :], 1.0 / WSCALE)
                    else:
                        V.tensor_scalar_mul(emb[:, i, :], pe[0:L, :],
                                            1.0 / WSCALE)
                nc.scalar.dma_start(
                    bass.AP(em_ds[q], 0, [[1024, L], [1, 1024]]), emb[:])
                nc.scalar.dma_start(
                    sl(emt, 32 * q, 32, 0, [[TS, L], [1, TS]]),
                    bass.AP(em_ds[q], 0, [[TS, 32], [1024, L], [1, TS]]))

            ga2 = gp.tile([128, 2], f32, name="ga2")  # [gold, ln-offset]

            # ---- exp of emissions (biases folded into the pair tables) ----
            em_e = rp.tile([128, L, TS], bf16)
            nc.scalar.activation(
                sl(em_e, 0, 128, 0, [[1, L * TS]]),
                sl(emt, 0, 128, 0, [[1, L * TS]]),
                Act.Exp)

            # ---- L0: j-mults on V (j=0,1) and Pool (j=2), adds + eb on V ----
            ta0 = rp.tile([128, 16, 9], bf16, name="ta0")
            tb0 = rp.tile([128, 16, 9], bf16, name="tb0")
            tg0 = rp.tile([128, 16, 9], bf16, name="tg0")
            r0 = rp.tile([128, 16, 9], bf16, name="r0")
            l0sl = lambda t: sl(t, 0, 128, 0, [[9, 16], [3, 3], [1, 3]])
            tab = lambda j: cbsl(0, 128, CPAIR + 9 * j,
                                 [[27, 16], [3, 3], [1, 3]])
            ea = lambda j: sl(em_e, 0, 128, 2 * 0 + TS * j,
                              [[2, 16], [0, 3], [0, 3]])
            V.tensor_mul(l0sl(ta0), tab(0), ea(0))
            V.tensor_mul(l0sl(tb0), tab(1), ea(1))
            V.tensor_mul(l0sl(tg0), tab(2), ea(2))
            V.tensor_add(l0sl(ta0), l0sl(ta0), l0sl(tb0))
            V.tensor_add(l0sl(ta0), l0sl(ta0), l0sl(tg0))
            V.tensor_mul(
                sl(r0, 0, 128, 0, [[9, 16], [3, 3], [1, 3]]),
                l0sl(ta0),
                sl(em_e, 0, 128, 1, [[2, 16], [0, 3], [TS, 3]]))

            # ---- binary fold: j-mults on V (j=0,1) and Pool (j=2), then one
            # V X-reduce over j (one dependence hop per level) ----
            def fold(cur, stride, pb, np_, n, tmp, nxt):
                half = n // 2
                A = lambda j: sl(cur, pb, np_, j,
                                 [[2 * stride, half], [3, 3], [0, 3]])
                Bp = lambda j: sl(cur, pb, np_, stride + 3 * j,
                                  [[2 * stride, half], [0, 3], [1, 3]])
                for j in range(3):
                    V.tensor_mul(
                        sl(tmp, pb, np_, j, [[27, half], [9, 3], [3, 3]]),
                        A(j), Bp(j))
                V.tensor_reduce(
                    sl(nxt, pb, np_, 0, [[9, half], [1, 9]]),
                    sl(tmp, pb, np_, 0, [[27, half], [3, 9], [1, 3]]),
                    axis=AX.X, op=Alu.add)

            lv8 = rp.tile([128, 8, 9], bf16, name="lv8")
            lv4 = rp.tile([128, 4, 9], bf16, name="lv4")
            lv2 = rp.tile([128, 2, 9], bf16, name="lv2")
            tmp8 = rp.tile([128, 8, 27], bf16, name="tmp8")
            r1 = rp.tile([128, 9], f32, name="r1")
            Sm = rp.tile([128, 27], bf16, name="Sm")

            with nc.allow_low_precision("bf16 tree folds (3-term adds)"):
                fold(r0, 9, 0, 128, 16, tmp8, lv8)
                fold(lv8, 9, 0, 128, 8, tmp8, lv4)
                fold(lv4, 9, 0, 128, 4, tmp8, lv2)
            # last in-partition fold via mult + X-reduce (f32 out)
            V.tensor_mul(
                sl(Sm, 0, 128, 0, [[9, 3], [3, 3], [1, 3]]),
                sl(lv2, 0, 128, 0, [[3, 3], [0, 3], [1, 3]]),
                sl(lv2, 0, 128, 9, [[0, 3], [1, 3], [3, 3]]))
            V.tensor_reduce(
                sl(r1, 0, 128, 0, [[3, 3], [1, 3]]),
                sl(Sm, 0, 128, 0, [[9, 3], [3, 3], [1, 3]]),
                axis=AX.X, op=Alu.add)

            # ---- normalize; ln(max) goes to the gather tile col 1 ----
            mx = rp.tile([128, 1], f32, name="mx")
            rinv = rp.tile([128, 1], f32, name="rinv")
            rec9 = rp.tile([128, 9], bf16, name="rec9")
            V.tensor_reduce(sl(mx, 0, 128, 0, [[1, 1]]),
                            sl(r1, 0, 128, 0, [[1, 9]]),
                            axis=AX.X, op=Alu.max)
            V.reciprocal(sl(rinv, 0, 128, 0, [[1, 1]]),
                         sl(mx, 0, 128, 0, [[1, 1]]))
            V.tensor_mul(
                sl(rec9, 0, 128, 0, [[1, 9]]),
                sl(r1, 0, 128, 0, [[1, 9]]),
                sl(rinv, 0, 128, 0, [[0, 9]]))
            nc.scalar.activation(sl(ga2, 0, 128, 1, [[1, 1]]),
                                 sl(mx, 0, 128, 0, [[1, 1]]), Act.Ln)

            # ---- gold emission-sum: Pool mult, scalar accum-copy ----
            dsc = gp.tile([128, L * TS], bf16, name="dsc")
            G.tensor_mul(
                sl(dsc, 0, 128, 0, [[1, L * TS]]),
                cbsl(0, 128, COH, [[1, L * TS]]),
                sl(emt, 0, 128, 0, [[1, L * TS]]))
            nc.scalar.activation(
                sl(dsc, 0, 128, 0, [[1, L * TS]]),
                sl(dsc, 0, 128, 0, [[1, L * TS]]),
                Act.Copy, accum_out=sl(ga2, 0, 128, 0, [[1, 1]]))

            # ---- per-seq gather: gold sum + ln-offset sum in one matmul ----
            sc8 = psp.tile([BC, 2], f32, name="sc8")
            nc.tensor.matmul(sc8[:], cfsl(0, 128, CSEL, [[1, BC]]),
                             sl(ga2, 0, 128, 0, [[1, 2]]),
                             start=True, stop=True)

            # ---- pack 16 chunk records per seq to partitions 0-7 (Pool
            # ring: the issue happens early, the descriptor waits) ----
            G.dma_start(
                sl(packT, 0, BC, 0, [[1, 144]]),
                sl(rec9, 0, 128, 0, [[1, 9]]))

            # ---- tail tree 16 -> 1 per seq; last level XY-reduces row 0
            # straight to Z ----
            t8 = gp.tile([128, 8, 9], bf16, name="t8")
            t4 = gp.tile([128, 4, 9], bf16, name="t4")
            t2 = gp.tile([128, 2, 9], bf16, name="t2")
            SmF = gp.tile([128, 9], bf16, name="SmF")
            zs = gp.tile([128, 1], f32, name="zs")
            with nc.allow_low_precision("bf16 tail folds (3-term adds)"):
                fold(packT, 9, 0, BC, 16, tmp8, t8)
                fold(t8, 9, 0, BC, 8, tmp8, t4)
                fold(t4, 9, 0, BC, 4, tmp8, t2)
            # zs = sum_{j,k} t2[rec0][0,j] * t2[rec1][j,k]  (end_transitions
            # folded into the last pair table on seq-end partitions)
            V.tensor_mul(
                sl(SmF, 0, BC, 0, [[3, 3], [1, 3]]),
                sl(t2, 0, BC, 0, [[1, 3], [0, 3]]),
                sl(t2, 0, BC, 9, [[3, 3], [1, 3]]))
            V.tensor_reduce(
                sl(zs, 0, BC, 0, [[1, 1]]),
                sl(SmF, 0, BC, 0, [[3, 3], [1, 3]]),
                axis=AX.XY, op=Alu.add)

            # ---- finals ----
            lz = gp.tile([128, 1], f32, name="lz")
            logz = gp.tile([128, 1], f32, name="logz")
            diffT = gp.tile([128, 1], f32, name="diffT")
            nc.scalar.activation(sl(lz, 0, BC, 0, [[1, 1]]),
                                 sl(zs, 0, BC, 0, [[1, 1]]), Act.Ln)
            V.tensor_add(sl(logz, 0, BC, 0, [[1, 1]]),
                         sl(lz, 0, BC, 0, [[1, 1]]),
                         sl(sc8, 0, BC, 1, [[1, 1]]))
            V.scalar_tensor_tensor(
                sl(diffT, 0, BC, 0, [[1, 1]]),
                sl(logz, 0, BC, 0, [[1, 1]]),
                cfsl(0, BC, CGC, [[1, 1]]),
                sl(sc8, 0, BC, 0, [[1, 1]]),
                Alu.subtract, Alu.subtract)
            G.dma_start(out[:], sl(diffT, 0, BC, 0, [[1, 1]]))

    nc.compile()
    return nc


_NC_CACHE = {}


def get_nc(debug=False):
    if "nc" not in _NC_CACHE:
        _NC_CACHE["nc"] = _build_nc(debug)
    return _NC_CACHE["nc"]


def make_in_maps(hidden, W, b, start_transitions, end_transitions, transitions,
                 attention_mask, labels):
    hidden = np.asarray(hidden, dtype=np.float32)
    W = np.asarray(W, dtype=np.float32)
    bb = np.asarray(b, dtype=np.float32)
    st = np.asarray(start_transitions, dtype=np.float32)
    en = np.asarray(end_transitions, dtype=np.float32)
    tr = np.asarray(transitions, dtype=np.float32)
    lab = np.asarray(labels)
    lab = np.where(lab < 0, 0, lab).astype(np.int64)

    e4 = ml_dtypes.float8_e4m3
    b16 = ml_dtypes.bfloat16
    # w8[p, (sc, i, l)] = W[sc*256 + i*128 + p, l] * 64, l padded 3 -> 16
    w8f = np.zeros((NSC, 2, 128, 16), dtype=np.float32)
    w8f[:, :, :, :L] = (W * WSCALE).reshape(NSC, 2, 128, L)
    w8 = np.ascontiguousarray(
        w8f.transpose(2, 0, 1, 3).reshape(128, NSC * 32)).astype(e4)

    baseb = np.zeros((128, NCB), dtype=np.float32)
    # pair tables [u][j][(i,k)]: exp(T_ij + b_j + T_jk + b_k); pair 0 on
    # seq-start partitions folds start_transitions (i-replicated); pair 15
    # on seq-end partitions folds end_transitions into the k leg
    u1 = np.exp(tr[:, :, None] + bb[None, :, None]
                + tr.T[None, :, :] + bb[None, None, :])       # [i, j, k]
    u1 = np.ascontiguousarray(u1.transpose(1, 0, 2)).reshape(27)  # [j,(i,k)]
    u0 = np.exp(st[:, None] + bb[:, None] + tr + bb[None, :])     # [j, k]
    u0 = np.broadcast_to(u0[:, None, :], (3, 3, 3)).reshape(27)   # i-bcast
    uE = np.exp(tr[:, :, None] + bb[None, :, None]
                + tr.T[None, :, :] + bb[None, None, :]
                + en[None, None, :])
    uE = np.ascontiguousarray(uE.transpose(1, 0, 2)).reshape(27)
    baseb[:, CPAIR:CPAIR + 432] = np.tile(u1, 16)
    baseb[0::NCH, CPAIR:CPAIR + 27] = u0
    baseb[NCH - 1::NCH, CPAIR + 27 * 15:CPAIR + 432] = uE

    basef = np.zeros((128, NCF), dtype=np.float32)
    p = np.arange(128)
    basef[:, CSEL:CSEL + BC] = (p[:, None] // NCH == np.arange(BC)[None, :])

    in_maps = []
    for c in range(NCORES):
        hc = hidden[c * BC:(c + 1) * BC]                      # [8, 512, 768]
        # hs[b, p, (sc, i, t)] = hidden[b, t, sc*256 + i*128 + p]
        hseq = np.ascontiguousarray(
            hc.reshape(BC, S, NSC, 2, 128).transpose(0, 4, 2, 3, 1)
        ).astype(e4).reshape(BC, 128, NSC * 1024)

        cbc = baseb.copy()
        labc = lab[c * BC:(c + 1) * BC]                       # [8, 512]
        # one-hot in tree layout [p=(s,c), l, t]
        labr = labc.reshape(128, TS)
        cbc[:, COH:COH + L * TS] = (
            labr[:, None, :] == np.arange(L)[None, :, None]
        ).astype(np.float32).reshape(128, L * TS)

        cfc = basef.copy()
        # per-seq gold constant: start + end + transition path + biases
        gc = (st[labc[:, 0]] + en[labc[:, -1]]
              + tr[labc[:, :-1], labc[:, 1:]].sum(axis=1)
              + bb[labc].sum(axis=1))
        cfc[:BC, CGC] = gc

        in_maps.append({
            "hseq": hseq,
            "w8": w8,
            "constb": cbc.astype(b16),
            "constf": cfc,
        })
    return in_maps


def kernel(hidden, W, b, start_transitions, end_transitions, transitions,
           attention_mask, labels):
    from concourse.bass_utils import run_bass_kernel_spmd

    nc = get_nc()
    in_maps = make_in_maps(hidden, W, b, start_transitions, end_transitions,
                           transitions, attention_mask, labels)
    res = run_bass_kernel_spmd(nc, in_maps, core_ids=list(range(NCORES)))
    total = 0.0
    for c in range(NCORES):
        total += float(res.results[c]["diff"].sum())
    return np.float32(total / B)


# revision 47
# speedup vs baseline: 1.1375x; 1.0829x over previous
"""CRF token-classifier loss (nn_CRFTokenClassifier) on 8 Trainium2 NeuronCores.

v5 strategy (data-parallel over batch, 8 sequences per core):
  - hidden staged fp8 (e4m3) in DoubleRow layout on the sync ring;
    emissions^T = (W*64)^T @ hidden^T as 24 dual-fp8 matmuls (K_eff=256,
    216ns/matmul at full clock). A ~6us dummy-weight PE warmup starting at
    preamble end lifts the HAM clock gate before the first pair lands
    (measured: first 14 matmuls run 630ns at half clock otherwise).
  - PSUM descaled (1/64) to SBUF bf16 by scalar+vector alternating;
    DRAM bounce (write then read, both on the scalar ring FIFO) into tree
    layout emt[p=(seq,chunk), l, 32] bf16.
  - log-partition on the vector engine in bf16 (2x DVE throughput): L0
    makes 16 pair records per partition in 3 instructions from a
    host-baked per-pair bf16 table exp(T_ij+b_j+T_jk+b_k);
    start_transitions folded into pair 0 of seq-start partitions,
    end_transitions into pair 15 of seq-end partitions; binary levels as
    3 independent strided mults + one X-reduce (4 instrs, one dependence
    hop); one max-normalize whose ln(max) rides the gold PE-gather
    (2-col rhs); single pack DMA to partitions 0-7; tail levels the same
    4-instr shape with the last level XY-reducing row 0 straight to Z.
  - gold score: host-baked bf16 one-hot labels, Pool mult + scalar
    accum-copy (f32 accum), summed per-seq by the PE gather; transition/
    start/end/bias terms are one host f32 scalar per sequence.
  - attention_mask is all ones by construction (fill: ones); masked-step
    handling is omitted like the baseline.
  - per-core output: per-sequence (logZ - score); host sums / B.
"""

import sys

if "/opt/trn_rl_repo" not in sys.path:
    sys.path.insert(0, "/opt/trn_rl_repo")

import numpy as np
import ml_dtypes

B, S, H, L = 64, 512, 768, 3
NCORES = 8
BC = B // NCORES            # 8 sequences (blocks) per core
NCH = 16                    # 32-step chunks per sequence
TS = 32
NPAIR = 4                   # block pairs
WSCALE = 64.0               # fp8 weight scale
NSC = 3                     # DoubleRow super-chunks (256 h each)

# bf16 const tensor column layout ([128, NCB])
CPAIR = 0                   # pair tables [u][j][(i,k)] : 16*27
COH = 432                   # one-hot labels [l,t] layout, 96 cols
NCB = 528
# f32 const tensor column layout ([128, NCF])
CSEL = 0                    # 8 cols: per-seq gather indicator
CGC = 8                     # 1 col: per-seq gold const at partitions 0-7
NCF = 9


def _patch_act_tables(arch):
    """Steer the act-table chooser so every activation we use resolves to the
    combined exp+ln set: one ACT_TABLE_LOAD for Copy / Exp / Ln."""
    from concourse.hw_specs import get_activation_tables
    from concourse import mybir

    A = mybir.ActivationFunctionType
    tabs = get_activation_tables(arch)
    combined = None
    for name, fns in tabs.items():
        if A.Exp in fns and A.Ln in fns:
            combined = name
            break
    if combined is None:
        return
    for f in (A.Exp, A.Ln, A.Copy, A.Identity):
        if f not in tabs[combined]:
            continue
        for name, fns in tabs.items():
            if name != combined:
                fns.discard(f)


def _build_nc(debug=False):
    import concourse.bass as bass
    import concourse.bacc as bacc
    import concourse.tile as tile
    from concourse import mybir

    f32 = mybir.dt.float32
    bf16 = mybir.dt.bfloat16
    fp8 = mybir.dt.float8e4
    Alu = mybir.AluOpType
    Act = mybir.ActivationFunctionType
    AX = mybir.AxisListType
    DR = mybir.MatmulPerfMode.DoubleRow

    nc = bacc.Bacc(None, target_bir_lowering=False, debug=debug)
    _patch_act_tables(nc.m.arch)

    hs_d = nc.dram_tensor("hseq", [BC, 128, NSC * 1024], fp8,
                          kind="ExternalInput")
    # stationary padded to 16 cols: dual-fp8 Ldweights ISA check rejects
    # narrow weight tiles (M=3/4 fail, M=16 passes)
    w_d = nc.dram_tensor("w8", [128, NSC * 32], fp8, kind="ExternalInput")
    cb_d = nc.dram_tensor("constb", [128, NCB], bf16, kind="ExternalInput")
    cf_d = nc.dram_tensor("constf", [128, NCF], f32, kind="ExternalInput")
    out = nc.dram_tensor("diff", [BC, 1], f32, kind="ExternalOutput")

    em_ds = [nc.dram_tensor(f"em_scratch{q}", [L, 1024], bf16)
             for q in range(NPAIR)]

    def sl(tile_h, pb, nparts, extra, dims):
        """AP over a tile's partitions [pb, pb+nparts), free-dim pattern
        `dims`, extra element offset `extra`."""
        ap = tile_h[:]
        return bass.AP(tile_h.tensor, ap.offset + pb * ap.ap[0][0] + extra,
                       [[ap.ap[0][0], nparts]] + dims)

    with tile.TileContext(nc) as tc:
        with (
            tc.tile_pool(name="consts", bufs=1) as cp,
            tc.tile_pool(name="hload", bufs=1) as hp,
            tc.tile_pool(name="emx", bufs=2) as ep,
            tc.tile_pool(name="tree", bufs=1) as rp,
            tc.tile_pool(name="gold", bufs=1) as gp,
            tc.tile_pool(name="pe", bufs=4, space="PSUM") as pep,
            tc.tile_pool(name="ps", bufs=1, space="PSUM") as psp,
        ):
            V = nc.vector
            G = nc.gpsimd

            # ---- PE warmup on memset dummy data (no DMA gate): the HAM
            # clock gate tracks ~6us of SUBSTANTIAL matmul work (tiny
            # matmuls don't move it), so run 9 full-width dummy matmuls
            # before pair 0 lands ----
            wdum = cp.tile([128, 16], fp8)
            wdumB = cp.tile([128, 512], fp8)
            G.memset(sl(wdum, 0, 128, 0, [[1, 16]]), 0.25)
            V.memset(sl(wdumB, 0, 128, 0, [[1, 512]]), 0.25)
            pwarm = psp.tile([16, 512], f32, name="pwarm")
            for _ in range(10):
                nc.tensor.matmul(pwarm[:], wdum[:], wdumB[:],
                                 start=True, stop=True)

            # ---- preloads: w8 first on the sync ring, then the hidden
            # pairs; const tensors on the scalar ring ----
            wsb = cp.tile([128, NSC, 2, 16], fp8)
            nc.sync.dma_start(
                wsb[:], w_d[:].rearrange("p (s i l) -> p s i l", i=2, l=16))
            cb = cp.tile([128, NCB], bf16)
            nc.scalar.dma_start(cb[:], cb_d[:])
            cf = cp.tile([128, NCF], f32)
            nc.scalar.dma_start(cf[:], cf_d[:])

            def cbsl(pb, nparts, col, dims):
                return sl(cb, pb, nparts, col, dims)

            def cfsl(pb, nparts, col, dims):
                return sl(cf, pb, nparts, col, dims)

            hs = hp.tile([128, BC, NSC * 1024], fp8)
            for q in range(NPAIR):
                nc.sync.dma_start(
                    hs[:, 2 * q:2 * q + 2, :],
                    hs_d[2 * q:2 * q + 2].rearrange("b p x -> p b x"))

            packT = gp.tile([128, 16, 9], bf16, name="packT")

            # ---- emissions: per pair, 6 DoubleRow matmuls (+12 filler
            # warmups to hold the clock through DMA gaps); the two descale
            # copies run on scalar+vector in parallel; bounce write on the
            # scalar ring (program order after its copy), read on the Pool
            # ring (pre-issued descriptor, waits the write semaphore) ----
            emt = rp.tile([128, L, TS], bf16)   # [p=(seq,chunk), l, t]
            for q in range(NPAIR):
                emb = ep.tile([L, 2, 512], bf16, tag="emb")
                for i in range(2):
                    b = 2 * q + i
                    pe = pep.tile([16, 512], f32, tag="pe")
                    for sc in range(NSC):
                        nc.tensor.matmul(
                            pe[:],
                            wsb[:, sc, :, :],
                            sl(hs, 0, 128, b * (NSC * 1024) + sc * 1024,
                               [[512, 2], [1, 512]]),
                            start=(sc == 0), stop=(sc == NSC - 1),
                            perf_mode=DR)
                    if i == 0:
                        nc.scalar.mul(emb[:, i, :], pe[0:L, :], 1.0 / WSCALE)
                    else:
                        V.tensor_scalar_mul(emb[:, i, :], pe[0:L, :],
                                            1.0 / WSCALE)
                nc.scalar.dma_start(
                    bass.AP(em_ds[q], 0, [[1024, L], [1, 1024]]), emb[:])
                G.dma_start(
                    sl(emt, 32 * q, 32, 0, [[TS, L], [1, TS]]),
                    bass.AP(em_ds[q], 0, [[TS, 32], [1024, L], [1, TS]]))

            ga2 = gp.tile([128, 2], f32, name="ga2")  # [gold, ln-offset]

            # ---- exp of emissions (biases folded into the pair tables) ----
            em_e = rp.tile([128, L, TS], bf16)
            nc.scalar.activation(
                sl(em_e, 0, 128, 0, [[1, L * TS]]),
                sl(emt, 0, 128, 0, [[1, L * TS]]),
                Act.Exp)

            # ---- L0: j-mults on V (j=0,1) and Pool (j=2), adds + eb on V ----
            ta0 = rp.tile([128, 16, 9], bf16, name="ta0")
            tb0 = rp.tile([128, 16, 9], bf16, name="tb0")
            tg0 = rp.tile([128, 16, 9], bf16, name="tg0")
            r0 = rp.tile([128, 16, 9], bf16, name="r0")
            l0sl = lambda t: sl(t, 0, 128, 0, [[9, 16], [3, 3], [1, 3]])
            tab = lambda j: cbsl(0, 128, CPAIR + 9 * j,
                                 [[27, 16], [3, 3], [1, 3]])
            ea = lambda j: sl(em_e, 0, 128, 2 * 0 + TS * j,
                              [[2, 16], [0, 3], [0, 3]])
            V.tensor_mul(l0sl(ta0), tab(0), ea(0))
            V.tensor_mul(l0sl(tb0), tab(1), ea(1))
            V.tensor_mul(l0sl(tg0), tab(2), ea(2))
            V.tensor_add(l0sl(ta0), l0sl(ta0), l0sl(tb0))
            V.tensor_add(l0sl(ta0), l0sl(ta0), l0sl(tg0))
            V.tensor_mul(
                sl(r0, 0, 128, 0, [[9, 16], [3, 3], [1, 3]]),
                l0sl(ta0),
                sl(em_e, 0, 128, 1, [[2, 16], [0, 3], [TS, 3]]))

            # ---- binary fold: j-mults on V (j=0,1) and Pool (j=2), then one
            # V X-reduce over j (one dependence hop per level) ----
            def fold(cur, stride, pb, np_, n, tmp, nxt):
                half = n // 2
                A = lambda j: sl(cur, pb, np_, j,
                                 [[2 * stride, half], [3, 3], [0, 3]])
                Bp = lambda j: sl(cur, pb, np_, stride + 3 * j,
                                  [[2 * stride, half], [0, 3], [1, 3]])
                for j in range(3):
                    V.tensor_mul(
                        sl(tmp, pb, np_, j, [[27, half], [9, 3], [3, 3]]),
                        A(j), Bp(j))
                V.tensor_reduce(
                    sl(nxt, pb, np_, 0, [[9, half], [1, 9]]),
                    sl(tmp, pb, np_, 0, [[27, half], [3, 9], [1, 3]]),
                    axis=AX.X, op=Alu.add)

            lv8 = rp.tile([128, 8, 9], bf16, name="lv8")
            lv4 = rp.tile([128, 4, 9], bf16, name="lv4")
            lv2 = rp.tile([128, 2, 9], bf16, name="lv2")
            tmp8 = rp.tile([128, 8, 27], bf16, name="tmp8")
            r1 = rp.tile([128, 9], f32, name="r1")
            Sm = rp.tile([128, 27], bf16, name="Sm")

            with nc.allow_low_precision("bf16 tree folds (3-term adds)"):
                fold(r0, 9, 0, 128, 16, tmp8, lv8)
                fold(lv8, 9, 0, 128, 8, tmp8, lv4)
                fold(lv4, 9, 0, 128, 4, tmp8, lv2)
            # last in-partition fold via mult + X-reduce (f32 out)
            V.tensor_mul(
                sl(Sm, 0, 128, 0, [[9, 3], [3, 3], [1, 3]]),
                sl(lv2, 0, 128, 0, [[3, 3], [0, 3], [1, 3]]),
                sl(lv2, 0, 128, 9, [[0, 3], [1, 3], [3, 3]]))
            V.tensor_reduce(
                sl(r1, 0, 128, 0, [[3, 3], [1, 3]]),
                sl(Sm, 0, 128, 0, [[9, 3], [3, 3], [1, 3]]),
                axis=AX.X, op=Alu.add)

            # ---- normalize; ln(max) goes to the gather tile col 1 ----
            mx = rp.tile([128, 1], f32, name="mx")
            rinv = rp.tile([128, 1], f32, name="rinv")
            rec9 = rp.tile([128, 9], bf16, name="rec9")
            V.tensor_reduce(sl(mx, 0, 128, 0, [[1, 1]]),
                            sl(r1, 0, 128, 0, [[1, 9]]),
                            axis=AX.X, op=Alu.max)
            V.reciprocal(sl(rinv, 0, 128, 0, [[1, 1]]),
                         sl(mx, 0, 128, 0, [[1, 1]]))
            V.tensor_mul(
                sl(rec9, 0, 128, 0, [[1, 9]]),
                sl(r1, 0, 128, 0, [[1, 9]]),
                sl(rinv, 0, 128, 0, [[0, 9]]))
            nc.scalar.activation(sl(ga2, 0, 128, 1, [[1, 1]]),
                                 sl(mx, 0, 128, 0, [[1, 1]]), Act.Ln)

            # ---- gold emission-sum: Pool mult, scalar accum-copy ----
            dsc = gp.tile([128, L * TS], bf16, name="dsc")
            G.tensor_mul(
                sl(dsc, 0, 128, 0, [[1, L * TS]]),
                cbsl(0, 128, COH, [[1, L * TS]]),
                sl(emt, 0, 128, 0, [[1, L * TS]]))
            nc.scalar.activation(
                sl(dsc, 0, 128, 0, [[1, L * TS]]),
                sl(dsc, 0, 128, 0, [[1, L * TS]]),
                Act.Copy, accum_out=sl(ga2, 0, 128, 0, [[1, 1]]))

            # ---- per-seq gather: gold sum + ln-offset sum in one matmul ----
            sc8 = psp.tile([BC, 2], f32, name="sc8")
            nc.tensor.matmul(sc8[:], cfsl(0, 128, CSEL, [[1, BC]]),
                             sl(ga2, 0, 128, 0, [[1, 2]]),
                             start=True, stop=True)

            # ---- pack 16 chunk records per seq to partitions 0-7 (Pool
            # ring: the issue happens early, the descriptor waits) ----
            G.dma_start(
                sl(packT, 0, BC, 0, [[1, 144]]),
                sl(rec9, 0, 128, 0, [[1, 9]]))

            # ---- tail tree 16 -> 1 per seq; last level XY-reduces row 0
            # straight to Z ----
            t8 = gp.tile([128, 8, 9], bf16, name="t8")
            t4 = gp.tile([128, 4, 9], bf16, name="t4")
            t2 = gp.tile([128, 2, 9], bf16, name="t2")
            SmF = gp.tile([128, 9], bf16, name="SmF")
            zs = gp.tile([128, 1], f32, name="zs")
            with nc.allow_low_precision("bf16 tail folds (3-term adds)"):
                fold(packT, 9, 0, BC, 16, tmp8, t8)
                fold(t8, 9, 0, BC, 8, tmp8, t4)
                fold(t4, 9, 0, BC, 4, tmp8, t2)
            # zs = sum_{j,k} t2[rec0][0,j] * t2[rec1][j,k]  (end_transitions
            # folded into the last pair table on seq-end partitions)
            V.tensor_mul(
                sl(SmF, 0, BC, 0, [[3, 3], [1, 3]]),
                sl(t2, 0, BC, 0, [[1, 3], [0, 3]]),
                sl(t2, 0, BC, 9, [[3, 3], [1, 3]]))
            V.tensor_reduce(
                sl(zs, 0, BC, 0, [[1, 1]]),
                sl(SmF, 0, BC, 0, [[3, 3], [1, 3]]),
                axis=AX.XY, op=Alu.add)

            # ---- finals ----
            lz = gp.tile([128, 1], f32, name="lz")
            logz = gp.tile([128, 1], f32, name="logz")
            diffT = gp.tile([128, 1], f32, name="diffT")
            nc.scalar.activation(sl(lz, 0, BC, 0, [[1, 1]]),
                                 sl(zs, 0, BC, 0, [[1, 1]]), Act.Ln)
            V.tensor_add(sl(logz, 0, BC, 0, [[1, 1]]),
                         sl(lz, 0, BC, 0, [[1, 1]]),
                         sl(sc8, 0, BC, 1, [[1, 1]]))
            V.scalar_tensor_tensor(
                sl(diffT, 0, BC, 0, [[1, 1]]),
                sl(logz, 0, BC, 0, [[1, 1]]),
                cfsl(0, BC, CGC, [[1, 1]]),
                sl(sc8, 0, BC, 0, [[1, 1]]),
                Alu.subtract, Alu.subtract)
            G.dma_start(out[:], sl(diffT, 0, BC, 0, [[1, 1]]))

    nc.compile()
    return nc


_NC_CACHE = {}


def get_nc(debug=False):
    if "nc" not in _NC_CACHE:
        _NC_CACHE["nc"] = _build_nc(debug)
    return _NC_CACHE["nc"]


def make_in_maps(hidden, W, b, start_transitions, end_transitions, transitions,
                 attention_mask, labels):
    hidden = np.asarray(hidden, dtype=np.float32)
    W = np.asarray(W, dtype=np.float32)
    bb = np.asarray(b, dtype=np.float32)
    st = np.asarray(start_transitions, dtype=np.float32)
    en = np.asarray(end_transitions, dtype=np.float32)
    tr = np.asarray(transitions, dtype=np.float32)
    lab = np.asarray(labels)
    lab = np.where(lab < 0, 0, lab).astype(np.int64)

    e4 = ml_dtypes.float8_e4m3
    b16 = ml_dtypes.bfloat16
    # w8[p, (sc, i, l)] = W[sc*256 + i*128 + p, l] * 64, l padded 3 -> 16
    w8f = np.zeros((NSC, 2, 128, 16), dtype=np.float32)
    w8f[:, :, :, :L] = (W * WSCALE).reshape(NSC, 2, 128, L)
    w8 = np.ascontiguousarray(
        w8f.transpose(2, 0, 1, 3).reshape(128, NSC * 32)).astype(e4)

    baseb = np.zeros((128, NCB), dtype=np.float32)
    # pair tables [u][j][(i,k)]: exp(T_ij + b_j + T_jk + b_k); pair 0 on
    # seq-start partitions folds start_transitions (i-replicated); pair 15
    # on seq-end partitions folds end_transitions into the k leg
    u1 = np.exp(tr[:, :, None] + bb[None, :, None]
                + tr.T[None, :, :] + bb[None, None, :])       # [i, j, k]
    u1 = np.ascontiguousarray(u1.transpose(1, 0, 2)).reshape(27)  # [j,(i,k)]
    u0 = np.exp(st[:, None] + bb[:, None] + tr + bb[None, :])     # [j, k]
    u0 = np.broadcast_to(u0[:, None, :], (3, 3, 3)).reshape(27)   # i-bcast
    uE = np.exp(tr[:, :, None] + bb[None, :, None]
                + tr.T[None, :, :] + bb[None, None, :]
                + en[None, None, :])
    uE = np.ascontiguousarray(uE.transpose(1, 0, 2)).reshape(27)
    baseb[:, CPAIR:CPAIR + 432] = np.tile(u1, 16)
    baseb[0::NCH, CPAIR:CPAIR + 27] = u0
    baseb[NCH - 1::NCH, CPAIR + 27 * 15:CPAIR + 432] = uE

    basef = np.zeros((128, NCF), dtype=np.float32)
    p = np.arange(128)
    basef[:, CSEL:CSEL + BC] = (p[:, None] // NCH == np.arange(BC)[None, :])

    in_maps = []
    for c in range(NCORES):
        hc = hidden[c * BC:(c + 1) * BC]                      # [8, 512, 768]
        # hs[b, p, (sc, i, t)] = hidden[b, t, sc*256 + i*128 + p]
        hseq = np.ascontiguousarray(
            hc.reshape(BC, S, NSC, 2, 128).transpose(0, 4, 2, 3, 1)
        ).astype(e4).reshape(BC, 128, NSC * 1024)

        cbc = baseb.copy()
        labc = lab[c * BC:(c + 1) * BC]                       # [8, 512]
        # one-hot in tree layout [p=(s,c), l, t]
        labr = labc.reshape(128, TS)
        cbc[:, COH:COH + L * TS] = (
            labr[:, None, :] == np.arange(L)[None, :, None]
        ).astype(np.float32).reshape(128, L * TS)

        cfc = basef.copy()
        # per-seq gold constant: start + end + transition path + biases
        gc = (st[labc[:, 0]] + en[labc[:, -1]]
              + tr[labc[:, :-1], labc[:, 1:]].sum(axis=1)
              + bb[labc].sum(axis=1))
        cfc[:BC, CGC] = gc

        in_maps.append({
            "hseq": hseq,
            "w8": w8,
            "constb": cbc.astype(b16),
            "constf": cfc,
        })
    return in_maps


def kernel(hidden, W, b, start_transitions, end_transitions, transitions,
           attention_mask, labels):
    from concourse.bass_utils import run_bass_kernel_spmd

    nc = get_nc()
    in_maps = make_in_maps(hidden, W, b, start_transitions, end_transitions,
                           transitions, attention_mask, labels)
    res = run_bass_kernel_spmd(nc, in_maps, core_ids=list(range(NCORES)))
    total = 0.0
    for c in range(NCORES):
        total += float(res.results[c]["diff"].sum())
    return np.float32(total / B)
